# revision 2
# baseline (speedup 1.0000x reference)
"""Trainium2 Bass kernel for nn_AllOutputsGRU.

Model: L=2 independent GRU layers over the SAME input x (ensemble style),
output = mean over layers of the full hidden-state sequence.

  xi   = einsum('tbf,lgf->ltbg', x, W_ih) + b_ih      (input projections)
  hs_l = scan over T of GRU cell with W_hh[l], b_hh[l]
  out  = mean_l hs_l                                   (T, B, H)

Sharding: 8 cores = 2 layers x 4 batch-groups (16 samples each). Every core
runs a fully independent scan (no collectives): per chunk of Tc=32 steps it
computes the input projection xi with large matmuls, then runs 32 sequential
GRU steps. Layout is "transposed": gates live on partitions (3H split into
12 m-tiles of 128) and batch is the moving/free dim, so the elementwise gate
math runs on all 128 DVE/ACT lanes.

Recurrent matmul: W_hh^T tiles (bf16, stationary, FWL) x h^T (bf16 moving),
accumulated in PSUM as gh^T (128, 12, 16). Host pre-transposes/casts all
operands and reassembles the output (host work is not part of exec time).
"""

import sys

import numpy as np

try:
    import concourse.bass as bass  # noqa: F401
except ImportError:
    sys.path.insert(0, "/opt/trn_rl_repo")

import concourse.bass as bass
import concourse.bacc as bacc
import concourse.mybir as mybir
import concourse.tile as tile
from concourse.bass import ds
from concourse.bass_utils import run_bass_kernel_spmd

import ml_dtypes

BF16 = ml_dtypes.bfloat16

# Problem sizes (hardcoded per task spec).
T, B, F, H, L = 1024, 64, 256, 512, 2
NCORES = 8
NBG = 4          # batch groups
Bc = B // NBG    # 16 samples per core
Tc = 32          # timesteps per chunk
NCHUNK = T // Tc         # 32
NPAIR = NCHUNK // 2      # 16 For_i iterations, 2 chunks each
KH = H // 128            # 4  k-chunks of the recurrent contraction
KF = F // 128            # 2  k-chunks of the input contraction
M3H = 3 * H // 128       # 12 m-tiles of the gate dimension
COLS = Tc * Bc           # 512 free columns per chunk
XT_COLS = T * Bc + 2 * COLS  # padded so prefetch of chunks 32/33 is in-bounds

FP32 = mybir.dt.float32
DBF16 = mybir.dt.bfloat16
AF = mybir.ActivationFunctionType


def build_nc():
    nc = bacc.Bacc("TRN2", target_bir_lowering=False, debug=False)

    xt_d = nc.declare_dram_parameter("xt", [KF, 128, XT_COLS], DBF16, isOutput=False)
    wih_d = nc.declare_dram_parameter("wih", [KF, 128, 3 * H], DBF16, isOutput=False)
    whh_d = nc.declare_dram_parameter("whh", [KH, 128, 3 * H], DBF16, isOutput=False)
    bias_d = nc.declare_dram_parameter("bias", [128, M3H], FP32, isOutput=False)
    bhn_d = nc.declare_dram_parameter("bhn", [128, KH, Bc], FP32, isOutput=False)
    out_d = nc.declare_dram_parameter("out", [KH, 128, T * Bc], FP32, isOutput=True)

    with tile.TileContext(nc) as tc:
        with (
            tc.tile_pool(name="const", bufs=1) as cpool,
            tc.tile_pool(name="xt", bufs=1) as xtpool,
            tc.tile_pool(name="xi", bufs=1) as xipool,
            tc.tile_pool(name="hs", bufs=1) as hspool,
            tc.tile_pool(name="tmp", bufs=2) as tmp,
            tc.tile_pool(name="ghp", bufs=2, space="PSUM") as ghpool,
            tc.tile_pool(name="xip", bufs=2, space="PSUM") as xippool,
        ):
            whh_t = cpool.tile([128, KH, 3 * H], DBF16, tag="whh")
            wih_t = cpool.tile([128, KF, 3 * H], DBF16, tag="wih")
            bias_t = cpool.tile([128, M3H], FP32, tag="bias")
            bhn_t = cpool.tile([128, KH, Bc], FP32, tag="bhn")
            h16 = [cpool.tile([128, KH, Bc], DBF16, tag=f"h16_{p}", name=f"h16_{p}") for p in range(2)]
            xt_t = [xtpool.tile([128, KF, COLS], DBF16, tag=f"xt_{p}", name=f"xt_{p}") for p in range(2)]
            xi_t = [xipool.tile([128, M3H, Tc, Bc], FP32, tag=f"xi_{p}", name=f"xi_{p}") for p in range(2)]
            hs_t = [hspool.tile([128, KH, Tc, Bc], FP32, tag=f"hs_{p}", name=f"hs_{p}") for p in range(2)]

            # Load weights/biases once.
            for k in range(KH):
                nc.sync.dma_start(whh_t[:, k, :], whh_d[k])
            for k in range(KF):
                nc.sync.dma_start(wih_t[:, k, :], wih_d[k])
            nc.sync.dma_start(bias_t[:], bias_d[:])
            nc.sync.dma_start(bhn_t[:, :, :], bhn_d[:])

            # h_{-1} = 0: zero the bf16 h16[1] and the f32 slot that global
            # step 0 reads (last column of hs buffer B).
            nc.vector.memset(h16[1][:, :, :], 0.0)
            nc.vector.memset(hs_t[1][:, :, Tc - 1, :], 0.0)

            def emit_xi(xt_buf, xi_buf):
                """xi_buf[m, t, b] = (x_chunk @ W_ih^T)[m-tile] + bias col m."""
                for m in range(M3H):
                    xp = xippool.tile([128, Tc, Bc], FP32, tag="xp")
                    for k in range(KF):
                        nc.tensor.matmul(
                            xp[:, :, :],
                            wih_t[:, k, m * 128:(m + 1) * 128],
                            xt_buf[:, k, :],
                            start=(k == 0),
                            stop=(k == KF - 1),
                        )
                    nc.vector.tensor_scalar_add(
                        xi_buf[:, m, :, :], xp[:, :, :], bias_t[:, m:m + 1]
                    )

            def emit_scan(xi_buf, hs_buf, hs_prev):
                """32 GRU steps; reads xi_buf, writes hs_buf (f32 h history)."""
                for s in range(Tc):
                    gh = ghpool.tile([128, M3H, Bc], FP32, tag="gh")
                    hin = h16[(s + 1) % 2]
                    for m in range(M3H):
                        for k in range(KH):
                            nc.tensor.matmul(
                                gh[:, m, :],
                                whh_t[:, k, m * 128:(m + 1) * 128],
                                hin[:, k, :],
                                start=(k == 0),
                                stop=(k == KH - 1),
                            )
                    sr = tmp.tile([128, KH, Bc], FP32, tag="sr")
                    nc.vector.tensor_add(sr[:], gh[:, 0:4, :], xi_buf[:, 0:4, s, :])
                    r = tmp.tile([128, KH, Bc], FP32, tag="r")
                    nc.scalar.activation(r[:], sr[:], AF.Sigmoid)
                    sz = tmp.tile([128, KH, Bc], FP32, tag="sz")
                    nc.vector.tensor_add(sz[:], gh[:, 4:8, :], xi_buf[:, 4:8, s, :])
                    z = tmp.tile([128, KH, Bc], FP32, tag="z")
                    nc.scalar.activation(z[:], sz[:], AF.Sigmoid)
                    gn = tmp.tile([128, KH, Bc], FP32, tag="gn")
                    nc.vector.tensor_add(gn[:], gh[:, 8:12, :], bhn_t[:, :, :])
                    t1 = tmp.tile([128, KH, Bc], FP32, tag="t1")
                    nc.vector.tensor_mul(t1[:], r[:], gn[:])
                    sn = tmp.tile([128, KH, Bc], FP32, tag="sn")
                    nc.vector.tensor_add(sn[:], t1[:], xi_buf[:, 8:12, s, :])
                    n = tmp.tile([128, KH, Bc], FP32, tag="n")
                    nc.scalar.activation(n[:], sn[:], AF.Tanh)
                    hprev = hs_prev[:, :, Tc - 1, :] if s == 0 else hs_buf[:, :, s - 1, :]
                    d = tmp.tile([128, KH, Bc], FP32, tag="d")
                    nc.vector.tensor_sub(d[:], hprev, n[:])
                    e = tmp.tile([128, KH, Bc], FP32, tag="e")
                    nc.vector.tensor_mul(e[:], z[:], d[:])
                    nc.vector.tensor_add(hs_buf[:, :, s, :], n[:], e[:])
                    nc.scalar.activation(h16[s % 2][:, :, :], hs_buf[:, :, s, :], AF.Copy)

            # Prologue: x(0) -> xtA, xi(0) -> xiA, x(1) -> xtB.
            for k in range(KF):
                nc.sync.dma_start(xt_t[0][:, k, :], xt_d[k, :, 0:COLS])
            emit_xi(xt_t[0], xi_t[0])
            for k in range(KF):
                nc.sync.dma_start(xt_t[1][:, k, :], xt_d[k, :, COLS:2 * COLS])

            HINTS = (mybir.EngineType.PE, mybir.EngineType.DVE, mybir.EngineType.Activation)
            with tc.For_i(0, NPAIR, 1, hint_engines=HINTS) as i:
                # chunks c0 = 2i (buffers A=0), c1 = 2i+1 (buffers B=1)
                emit_scan(xi_t[0], hs_t[0], hs_t[1])
                emit_xi(xt_t[1], xi_t[1])          # xi(c1), fills scan-A bubbles
                for hc in range(KH):
                    nc.sync.dma_start(
                        out_d[hc, :, ds(i * (2 * COLS), COLS)], hs_t[0][:, hc, :, :]
                    )
                for k in range(KF):                 # x(c0+2) -> xtA
                    nc.sync.dma_start(
                        xt_t[0][:, k, :], xt_d[k, :, ds(i * (2 * COLS) + 2 * COLS, COLS)]
                    )
                emit_scan(xi_t[1], hs_t[1], hs_t[0])
                emit_xi(xt_t[0], xi_t[0])          # xi(c0+2), fills scan-B bubbles
                for hc in range(KH):
                    nc.sync.dma_start(
                        out_d[hc, :, ds(i * (2 * COLS) + COLS, COLS)], hs_t[1][:, hc, :, :]
                    )
                for k in range(KF):                 # x(c1+2) -> xtB
                    nc.sync.dma_start(
                        xt_t[1][:, k, :], xt_d[k, :, ds(i * (2 * COLS) + 3 * COLS, COLS)]
                    )

    nc.compile()
    return nc


_NC_CACHE = None


def _get_nc():
    global _NC_CACHE
    if _NC_CACHE is None:
        _NC_CACHE = build_nc()
    return _NC_CACHE


def _prep_core_inputs(x, W_ih, W_hh, b_ih, b_hh, layer, bg):
    xs = x[:, bg * Bc:(bg + 1) * Bc, :]                   # (T, Bc, F)
    xt = np.ascontiguousarray(np.transpose(xs, (2, 0, 1)))  # (F, T, Bc)
    xt = xt.reshape(KF, 128, T * Bc)
    xt_p = np.zeros((KF, 128, XT_COLS), np.float32)
    xt_p[:, :, :T * Bc] = xt

    wih = np.ascontiguousarray(W_ih[layer].T).reshape(KF, 128, 3 * H)
    whh = np.ascontiguousarray(W_hh[layer].T).reshape(KH, 128, 3 * H)

    bias_full = b_ih[layer].copy()
    bias_full[:2 * H] += b_hh[layer][:2 * H]
    bias = np.ascontiguousarray(bias_full.reshape(M3H, 128).T)  # (128, 12)

    bhn = b_hh[layer][2 * H:].reshape(KH, 128).T          # (128, KH)
    bhn = np.ascontiguousarray(
        np.broadcast_to(bhn[:, :, None], (128, KH, Bc))
    ).astype(np.float32)

    return {
        "xt": xt_p.astype(BF16),
        "wih": wih.astype(BF16),
        "whh": whh.astype(BF16),
        "bias": bias.astype(np.float32),
        "bhn": bhn,
    }


def run_cores(x, W_ih, W_hh, b_ih, b_hh, trace=False):
    nc = _get_nc()
    in_maps = [
        _prep_core_inputs(x, W_ih, W_hh, b_ih, b_hh, core // NBG, core % NBG)
        for core in range(NCORES)
    ]
    return run_bass_kernel_spmd(nc, in_maps, core_ids=list(range(NCORES)), trace=trace)


def assemble(results):
    out = np.zeros((T, B, H), np.float32)
    for bg in range(NBG):
        acc = None
        for layer in range(L):
            o = np.asarray(results[layer * NBG + bg]["out"], np.float32)
            hs = o.reshape(KH, 128, T, Bc).transpose(2, 3, 0, 1).reshape(T, Bc, H)
            acc = hs if acc is None else acc + hs
        out[:, bg * Bc:(bg + 1) * Bc, :] = acc / L
    return out


def kernel(x, W_ih, W_hh, b_ih, b_hh):
    x = np.asarray(x, np.float32)
    W_ih = np.asarray(W_ih, np.float32)
    W_hh = np.asarray(W_hh, np.float32)
    b_ih = np.asarray(b_ih, np.float32)
    b_hh = np.asarray(b_hh, np.float32)
    res = run_cores(x, W_ih, W_hh, b_ih, b_hh, trace=False)
    return assemble(res.results)


# revision 4
# speedup vs baseline: 1.0955x; 1.0955x over previous
"""Trainium2 Bass kernel for nn_AllOutputsGRU.

Model: L=2 independent GRU layers over the SAME input x (ensemble style),
output = mean over layers of the full hidden-state sequence (T, B, H).

Sharding: 8 cores = 2 layers x 4 batch-groups (16 samples each); every core
runs a fully independent scan (no collectives). Transposed layout: gates on
partitions (3H -> 12 m-tiles of 128), batch is the moving dim.

Per step, everything that can be an accumulating matmul is one:
  gh_rz (PSUM, 8x16) = W_hh_rz h + W_ih_rz x_t + b  (h-MMs + x-MMs + K=1 bias MM)
  gh_n  (PSUM, 4x16) = W_hh_n  h + b_hn             (h-MMs + K=1 bias MM)
so the r/z path is just: sigmoid(PSUM) -> DVE chain. The n-gate input
projection xi_n (+b_in) is precomputed per 32-step chunk with big matmuls.
bf16 weights/moving operands (FWL), fp32 PSUM accumulate, fp32 h state.
"""

import sys

import numpy as np

try:
    import concourse.bass as bass  # noqa: F401
except ImportError:
    sys.path.insert(0, "/opt/trn_rl_repo")

import concourse.bass as bass
import concourse.bacc as bacc
import concourse.mybir as mybir
import concourse.tile as tile
from concourse.bass import ds
from concourse.bass_utils import run_bass_kernel_spmd

import ml_dtypes

BF16 = ml_dtypes.bfloat16

# Problem sizes (hardcoded per task spec).
T, B, F, H, L = 1024, 64, 256, 512, 2
NCORES = 8
NBG = 4          # batch groups
Bc = B // NBG    # 16 samples per core
Tc = 32          # timesteps per chunk
NCHUNK = T // Tc         # 32
NPAIR = NCHUNK // 2      # 16 For_i iterations, 2 chunks each
KH = H // 128            # 4  k-chunks of the recurrent contraction
KF = F // 128            # 2  k-chunks of the input contraction
MRZ = 2 * H // 128       # 8  m-tiles for r,z gates
MN = H // 128            # 4  m-tiles for the n gate
COLS = Tc * Bc           # 512 free columns per chunk
XT_COLS = T * Bc + 2 * COLS  # padded so prefetch of chunks 32/33 is in-bounds

FP32 = mybir.dt.float32
DBF16 = mybir.dt.bfloat16
AF = mybir.ActivationFunctionType
ALU = mybir.AluOpType


def build_nc():
    nc = bacc.Bacc("TRN2", target_bir_lowering=False, debug=False)

    xt_d = nc.declare_dram_parameter("xt", [KF, 128, XT_COLS], DBF16, isOutput=False)
    wih_d = nc.declare_dram_parameter("wih", [KF, 128, 3 * H], DBF16, isOutput=False)
    whh_d = nc.declare_dram_parameter("whh", [KH, 128, 3 * H], DBF16, isOutput=False)
    brz_d = nc.declare_dram_parameter("brz", [1, MRZ, 128], DBF16, isOutput=False)
    bn_d = nc.declare_dram_parameter("bn", [1, MN, 128], DBF16, isOutput=False)
    bhn_d = nc.declare_dram_parameter("bhn", [1, MN, 128], DBF16, isOutput=False)
    out_d = nc.declare_dram_parameter("out", [KH, 128, T * Bc], FP32, isOutput=True)

    with tile.TileContext(nc) as tc:
        with (
            tc.tile_pool(name="const", bufs=1) as cpool,
            tc.tile_pool(name="xt", bufs=1) as xtpool,
            tc.tile_pool(name="xi", bufs=1) as xipool,
            tc.tile_pool(name="hs", bufs=1) as hspool,
            tc.tile_pool(name="tmp", bufs=2) as tmp,
            tc.tile_pool(name="grz", bufs=2, space="PSUM") as grzpool,
            tc.tile_pool(name="gn", bufs=2, space="PSUM") as gnpool,
            tc.tile_pool(name="xip", bufs=2, space="PSUM") as xippool,
        ):
            whh_t = cpool.tile([128, KH, 3 * H], DBF16, tag="whh")
            wih_t = cpool.tile([128, KF, 3 * H], DBF16, tag="wih")
            brz_t = cpool.tile([1, MRZ, 128], DBF16, tag="brz")
            bn_t = cpool.tile([1, MN, 128], DBF16, tag="bn")
            bhn_t = cpool.tile([1, MN, 128], DBF16, tag="bhn")
            ones16 = cpool.tile([1, Bc], DBF16, tag="ones16")
            ones512 = cpool.tile([1, COLS], DBF16, tag="ones512")
            h16 = [cpool.tile([128, KH, Bc], DBF16, tag=f"h16_{p}", name=f"h16_{p}") for p in range(2)]
            xt_t = [xtpool.tile([128, KF, COLS], DBF16, tag=f"xt_{p}", name=f"xt_{p}") for p in range(2)]
            xi_t = [xipool.tile([128, MN, Tc, Bc], FP32, tag=f"xi_{p}", name=f"xi_{p}") for p in range(2)]
            hs_t = [hspool.tile([128, KH, Tc, Bc], FP32, tag=f"hs_{p}", name=f"hs_{p}") for p in range(2)]

            # Load weights/biases once.
            for k in range(KH):
                nc.sync.dma_start(whh_t[:, k, :], whh_d[k])
            for k in range(KF):
                nc.sync.dma_start(wih_t[:, k, :], wih_d[k])
            nc.sync.dma_start(brz_t[:], brz_d[:])
            nc.sync.dma_start(bn_t[:], bn_d[:])
            nc.sync.dma_start(bhn_t[:], bhn_d[:])
            nc.vector.memset(ones16[:], 1.0)
            nc.vector.memset(ones512[:], 1.0)

            # h_{-1} = 0: zero the bf16 h16[1] and the f32 slot that global
            # step 0 reads (last column of hs buffer B).
            nc.vector.memset(h16[1][:, :, :], 0.0)
            nc.vector.memset(hs_t[1][:, :, Tc - 1, :], 0.0)

            def emit_xi(xt_buf, xi_buf):
                """xi_buf[m,t,b] = (x_chunk @ W_ih_n^T)[m] + b_in  (n-gate only)."""
                for m in range(MN):
                    xp = xippool.tile([128, Tc, Bc], FP32, tag="xp")
                    nc.tensor.matmul(xp[:], bn_t[:, m, :], ones512[:], start=True, stop=False)
                    for k in range(KF):
                        nc.tensor.matmul(
                            xp[:],
                            wih_t[:, k, (MRZ + m) * 128:(MRZ + m + 1) * 128],
                            xt_buf[:, k, :],
                            start=False,
                            stop=(k == KF - 1),
                        )
                    nc.scalar.activation(xi_buf[:, m, :, :], xp[:], AF.Copy)

            def emit_scan(xt_buf, xi_buf, hs_buf, hs_prev):
                """32 GRU steps; reads xt/xi, writes hs_buf (f32 h history)."""
                for s in range(Tc):
                    grz = grzpool.tile([128, MRZ, Bc], FP32, tag="grz")
                    gn = gnpool.tile([128, MN, Bc], FP32, tag="gn")
                    hin = h16[(s + 1) % 2]
                    # r/z gates: x-projection + bias first (independent of h,
                    # can run during the previous step's tail), then h-MMs.
                    # One accumulation group per PSUM bank: start=True only on
                    # the first MM of the tile (clears has_written for the
                    # whole bank), stop=True only on the last.
                    for m in range(MRZ):
                        nc.tensor.matmul(grz[:, m, :], brz_t[:, m, :], ones16[:],
                                         start=(m == 0), stop=False)
                        for k in range(KF):
                            nc.tensor.matmul(
                                grz[:, m, :],
                                wih_t[:, k, m * 128:(m + 1) * 128],
                                xt_buf[:, k, s * Bc:(s + 1) * Bc],
                                start=False, stop=False,
                            )
                    for m in range(MRZ):
                        for k in range(KH):
                            nc.tensor.matmul(
                                grz[:, m, :],
                                whh_t[:, k, m * 128:(m + 1) * 128],
                                hin[:, k, :],
                                start=False,
                                stop=(m == MRZ - 1 and k == KH - 1),
                            )
                    # n gate: bias + h-MMs into its own PSUM bank.
                    for m in range(MN):
                        nc.tensor.matmul(gn[:, m, :], bhn_t[:, m, :], ones16[:],
                                         start=(m == 0), stop=False)
                        for k in range(KH):
                            nc.tensor.matmul(
                                gn[:, m, :],
                                whh_t[:, k, (MRZ + m) * 128:(MRZ + m + 1) * 128],
                                hin[:, k, :],
                                start=False,
                                stop=(m == MN - 1 and k == KH - 1),
                            )
                    rz = tmp.tile([128, MRZ, Bc], FP32, tag="rz")
                    nc.scalar.activation(rz[:], grz[:], AF.Sigmoid)
                    t1 = tmp.tile([128, MN, Bc], FP32, tag="t1")
                    nc.vector.tensor_mul(t1[:], rz[:, 0:4, :], gn[:])
                    sn = tmp.tile([128, MN, Bc], FP32, tag="sn")
                    nc.vector.tensor_add(sn[:], t1[:], xi_buf[:, :, s, :])
                    n = tmp.tile([128, MN, Bc], FP32, tag="n")
                    nc.scalar.activation(n[:], sn[:], AF.Tanh)
                    # off-critical-path: u = z*h_prev ; oz = 1 - z
                    hprev = hs_prev[:, :, Tc - 1, :] if s == 0 else hs_buf[:, :, s - 1, :]
                    u = tmp.tile([128, MN, Bc], FP32, tag="u")
                    nc.vector.tensor_mul(u[:], rz[:, 4:8, :], hprev)
                    oz = tmp.tile([128, MN, Bc], FP32, tag="oz")
                    nc.vector.tensor_scalar(oz[:], rz[:, 4:8, :], -1.0, 1.0, ALU.mult, ALU.add)
                    # h' = oz*n + u : bf16 copy feeds the next matmul sweep,
                    # f32 copy (gpsimd) is the carried state / output.
                    v = tmp.tile([128, MN, Bc], FP32, tag="v")
                    nc.vector.tensor_mul(v[:], oz[:], n[:])
                    nc.vector.tensor_add(h16[s % 2][:, :, :], v[:], u[:])
                    nc.gpsimd.tensor_add(hs_buf[:, :, s, :], v[:], u[:])

            # Prologue: x(0) -> xtA, xi(0) -> xiA, x(1) -> xtB.
            for k in range(KF):
                nc.sync.dma_start(xt_t[0][:, k, :], xt_d[k, :, 0:COLS])
            emit_xi(xt_t[0], xi_t[0])
            for k in range(KF):
                nc.sync.dma_start(xt_t[1][:, k, :], xt_d[k, :, COLS:2 * COLS])

            HINTS = (mybir.EngineType.PE, mybir.EngineType.DVE, mybir.EngineType.Activation)
            with tc.For_i(0, NPAIR, 1, hint_engines=HINTS) as i:
                # chunks c0 = 2i (buffers A=0), c1 = 2i+1 (buffers B=1)
                emit_scan(xt_t[0], xi_t[0], hs_t[0], hs_t[1])
                emit_xi(xt_t[1], xi_t[1])          # xi(c1), fills scan-A bubbles
                for hc in range(KH):
                    nc.sync.dma_start(
                        out_d[hc, :, ds(i * (2 * COLS), COLS)], hs_t[0][:, hc, :, :]
                    )
                for k in range(KF):                 # x(c0+2) -> xtA
                    nc.sync.dma_start(
                        xt_t[0][:, k, :], xt_d[k, :, ds(i * (2 * COLS) + 2 * COLS, COLS)]
                    )
                emit_scan(xt_t[1], xi_t[1], hs_t[1], hs_t[0])
                emit_xi(xt_t[0], xi_t[0])          # xi(c0+2), fills scan-B bubbles
                for hc in range(KH):
                    nc.sync.dma_start(
                        out_d[hc, :, ds(i * (2 * COLS) + COLS, COLS)], hs_t[1][:, hc, :, :]
                    )
                for k in range(KF):                 # x(c1+2) -> xtB
                    nc.sync.dma_start(
                        xt_t[1][:, k, :], xt_d[k, :, ds(i * (2 * COLS) + 3 * COLS, COLS)]
                    )

    nc.compile()
    return nc


_NC_CACHE = None


def _get_nc():
    global _NC_CACHE
    if _NC_CACHE is None:
        _NC_CACHE = build_nc()
    return _NC_CACHE


def _prep_core_inputs(x, W_ih, W_hh, b_ih, b_hh, layer, bg):
    xs = x[:, bg * Bc:(bg + 1) * Bc, :]                   # (T, Bc, F)
    xt = np.ascontiguousarray(np.transpose(xs, (2, 0, 1)))  # (F, T, Bc)
    xt = xt.reshape(KF, 128, T * Bc)
    xt_p = np.zeros((KF, 128, XT_COLS), np.float32)
    xt_p[:, :, :T * Bc] = xt

    wih = np.ascontiguousarray(W_ih[layer].T).reshape(KF, 128, 3 * H)
    whh = np.ascontiguousarray(W_hh[layer].T).reshape(KH, 128, 3 * H)

    brz = (b_ih[layer][:2 * H] + b_hh[layer][:2 * H]).reshape(1, MRZ, 128)
    bn = b_ih[layer][2 * H:].reshape(1, MN, 128)
    bhn = b_hh[layer][2 * H:].reshape(1, MN, 128)

    return {
        "xt": xt_p.astype(BF16),
        "wih": wih.astype(BF16),
        "whh": whh.astype(BF16),
        "brz": brz.astype(BF16),
        "bn": bn.astype(BF16),
        "bhn": bhn.astype(BF16),
    }


def run_cores(x, W_ih, W_hh, b_ih, b_hh, trace=False):
    nc = _get_nc()
    in_maps = [
        _prep_core_inputs(x, W_ih, W_hh, b_ih, b_hh, core // NBG, core % NBG)
        for core in range(NCORES)
    ]
    return run_bass_kernel_spmd(nc, in_maps, core_ids=list(range(NCORES)), trace=trace)


def assemble(results):
    out = np.zeros((T, B, H), np.float32)
    for bg in range(NBG):
        acc = None
        for layer in range(L):
            o = np.asarray(results[layer * NBG + bg]["out"], np.float32)
            hs = o.reshape(KH, 128, T, Bc).transpose(2, 3, 0, 1).reshape(T, Bc, H)
            acc = hs if acc is None else acc + hs
        out[:, bg * Bc:(bg + 1) * Bc, :] = acc / L
    return out


def kernel(x, W_ih, W_hh, b_ih, b_hh):
    x = np.asarray(x, np.float32)
    W_ih = np.asarray(W_ih, np.float32)
    W_hh = np.asarray(W_hh, np.float32)
    b_ih = np.asarray(b_ih, np.float32)
    b_hh = np.asarray(b_hh, np.float32)
    res = run_cores(x, W_ih, W_hh, b_ih, b_hh, trace=False)
    return assemble(res.results)


# revision 5
# speedup vs baseline: 1.6939x; 1.5462x over previous
"""Trainium2 Bass kernel for nn_AllOutputsGRU.

Model: L=2 independent GRU layers over the SAME input x (ensemble style),
output = mean over layers of the full hidden-state sequence (T, B, H).

Sharding: 8 cores = 2 layers x 4 batch-groups (16 samples each); every core
runs a fully independent scan (no collectives). Transposed layout: gates on
partitions (3H -> 12 m-tiles of 128), batch is the moving dim.

Per step, everything that can be an accumulating matmul is one:
  gh_rz (PSUM, 8x16) = W_hh_rz h + W_ih_rz x_t + b  (h-MMs + x-MMs + K=1 bias MM)
  gh_n  (PSUM, 4x16) = W_hh_n  h + b_hn             (h-MMs + K=1 bias MM)
so the r/z path is just: sigmoid(PSUM) -> DVE chain. The n-gate input
projection xi_n (+b_in) is precomputed per 32-step chunk with big matmuls.
bf16 weights/moving operands (FWL), fp32 PSUM accumulate, fp32 h state.
"""

import sys

import numpy as np

try:
    import concourse.bass as bass  # noqa: F401
except ImportError:
    sys.path.insert(0, "/opt/trn_rl_repo")

import concourse.bass as bass
import concourse.bacc as bacc
import concourse.mybir as mybir
import concourse.tile as tile
from concourse.bass import ds
from concourse.bass_utils import run_bass_kernel_spmd

import ml_dtypes

BF16 = ml_dtypes.bfloat16

# Problem sizes (hardcoded per task spec).
T, B, F, H, L = 1024, 64, 256, 512, 2
NCORES = 8
NBG = 4          # batch groups
Bc = B // NBG    # 16 samples per core
Tc = 32          # timesteps per chunk
NCHUNK = T // Tc         # 32
NPAIR = NCHUNK // 2      # 16 For_i iterations, 2 chunks each
KH = H // 128            # 4  k-chunks of the recurrent contraction
KF = F // 128            # 2  k-chunks of the input contraction
MRZ = 2 * H // 128       # 8  m-tiles for r,z gates
MN = H // 128            # 4  m-tiles for the n gate
COLS = Tc * Bc           # 512 free columns per chunk
XT_COLS = T * Bc + 2 * COLS  # padded so prefetch of chunks 32/33 is in-bounds

FP32 = mybir.dt.float32
DBF16 = mybir.dt.bfloat16
AF = mybir.ActivationFunctionType
ALU = mybir.AluOpType


def build_nc():
    nc = bacc.Bacc("TRN2", target_bir_lowering=False, debug=False)

    xt_d = nc.declare_dram_parameter("xt", [KF, 128, XT_COLS], DBF16, isOutput=False)
    wih_d = nc.declare_dram_parameter("wih", [KF, 128, 3 * H], DBF16, isOutput=False)
    whh_d = nc.declare_dram_parameter("whh", [KH, 128, 3 * H], DBF16, isOutput=False)
    iden_d = nc.declare_dram_parameter("iden", [128, 128], DBF16, isOutput=False)
    bhnb_d = nc.declare_dram_parameter("bhnb", [128, MN, Bc], DBF16, isOutput=False)
    bias_d = nc.declare_dram_parameter("bias", [128, 3 * H // 128], FP32, isOutput=False)
    out_d = nc.declare_dram_parameter("out", [KH, 128, T * Bc], FP32, isOutput=True)

    with tile.TileContext(nc) as tc:
        with (
            tc.tile_pool(name="const", bufs=1) as cpool,
            tc.tile_pool(name="xt", bufs=1) as xtpool,
            tc.tile_pool(name="xi", bufs=1) as xipool,
            tc.tile_pool(name="hs", bufs=1) as hspool,
            tc.tile_pool(name="tmp", bufs=2) as tmp,
            tc.tile_pool(name="grz", bufs=2, space="PSUM") as grzpool,
            tc.tile_pool(name="gn", bufs=2, space="PSUM") as gnpool,
            tc.tile_pool(name="xip", bufs=2, space="PSUM") as xippool,
        ):
            whh_t = cpool.tile([128, KH, 3 * H], DBF16, tag="whh")
            wih_t = cpool.tile([128, KF, 3 * H], DBF16, tag="wih")
            iden_t = cpool.tile([128, 128], DBF16, tag="iden")
            bhnb_t = cpool.tile([128, MN, Bc], DBF16, tag="bhnb")
            bias_t = cpool.tile([128, 3 * H // 128], FP32, tag="bias")
            h16 = [cpool.tile([128, KH, Bc], DBF16, tag=f"h16_{p}", name=f"h16_{p}") for p in range(2)]
            xt_t = [xtpool.tile([128, KF, COLS], DBF16, tag=f"xt_{p}", name=f"xt_{p}") for p in range(2)]
            xi_t = [xipool.tile([128, 3 * H // 128, Tc, Bc], DBF16, tag=f"xi_{p}", name=f"xi_{p}") for p in range(2)]
            hs_t = [hspool.tile([128, KH, Tc, Bc], FP32, tag=f"hs_{p}", name=f"hs_{p}") for p in range(2)]

            # Load weights/biases once.
            for k in range(KH):
                nc.sync.dma_start(whh_t[:, k, :], whh_d[k])
            for k in range(KF):
                nc.sync.dma_start(wih_t[:, k, :], wih_d[k])
            nc.sync.dma_start(iden_t[:], iden_d[:])
            nc.sync.dma_start(bhnb_t[:, :, :], bhnb_d[:])
            nc.sync.dma_start(bias_t[:], bias_d[:])

            # h_{-1} = 0: zero the bf16 h16[1] and the f32 slot that global
            # step 0 reads (last column of hs buffer B).
            nc.vector.memset(h16[1][:, :, :], 0.0)
            nc.vector.memset(hs_t[1][:, :, Tc - 1, :], 0.0)

            def emit_xi(xt_buf, xi_buf):
                """xi_buf[m,t,b] = (x_chunk @ W_ih^T)[m] + bias[m]  (all gates;
                r/z bias includes b_hh since those add linearly)."""
                for m in range(3 * H // 128):
                    xp = xippool.tile([128, Tc, Bc], FP32, tag="xp")
                    for k in range(KF):
                        nc.tensor.matmul(
                            xp[:],
                            wih_t[:, k, m * 128:(m + 1) * 128],
                            xt_buf[:, k, :],
                            start=(k == 0),
                            stop=(k == KF - 1),
                        )
                    nc.scalar.activation(
                        xi_buf[:, m, :, :], xp[:], AF.Identity,
                        bias=bias_t[:, m:m + 1], scale=1.0,
                    )

            def emit_scan(xi_buf, hs_buf, hs_prev):
                """32 GRU steps; reads xt/xi, writes hs_buf (f32 h history)."""
                for s in range(Tc):
                    grz = grzpool.tile([128, MRZ, Bc], FP32, tag="grz")
                    gn = gnpool.tile([128, MN, Bc], FP32, tag="gn")
                    hin = h16[(s + 1) % 2]
                    # r/z gates: x-projection + bias first (independent of h,
                    # can run during the previous step's tail), then h-MMs.
                    # One accumulation group per PSUM bank; the group opens
                    # with an identity-matmul that injects the precomputed
                    # input projection (xi, bias included), then h-MMs
                    # accumulate on top.  No K=1 matmuls (they break FWL).
                    nc.tensor.matmul(grz[:, :, :], iden_t[:],
                                     xi_buf[:, 0:MRZ, s, :], start=True, stop=False)
                    for m in range(MRZ):
                        for k in range(KH):
                            nc.tensor.matmul(
                                grz[:, m, :],
                                whh_t[:, k, m * 128:(m + 1) * 128],
                                hin[:, k, :],
                                start=False,
                                stop=(m == MRZ - 1 and k == KH - 1),
                            )
                    # n gate: inject broadcast b_hn, then h-MMs.
                    nc.tensor.matmul(gn[:, :, :], iden_t[:],
                                     bhnb_t[:, :, :], start=True, stop=False)
                    for m in range(MN):
                        for k in range(KH):
                            nc.tensor.matmul(
                                gn[:, m, :],
                                whh_t[:, k, (MRZ + m) * 128:(MRZ + m + 1) * 128],
                                hin[:, k, :],
                                start=False,
                                stop=(m == MN - 1 and k == KH - 1),
                            )
                    rz = tmp.tile([128, MRZ, Bc], FP32, tag="rz")
                    nc.scalar.activation(rz[:], grz[:], AF.Sigmoid)
                    t1 = tmp.tile([128, MN, Bc], FP32, tag="t1")
                    nc.vector.tensor_mul(t1[:], rz[:, 0:4, :], gn[:])
                    sn = tmp.tile([128, MN, Bc], FP32, tag="sn")
                    nc.vector.tensor_add(sn[:], t1[:], xi_buf[:, MRZ:, s, :])
                    n = tmp.tile([128, MN, Bc], FP32, tag="n")
                    nc.scalar.activation(n[:], sn[:], AF.Tanh)
                    # off-critical-path: u = z*h_prev ; oz = 1 - z
                    hprev = hs_prev[:, :, Tc - 1, :] if s == 0 else hs_buf[:, :, s - 1, :]
                    u = tmp.tile([128, MN, Bc], FP32, tag="u")
                    nc.vector.tensor_mul(u[:], rz[:, 4:8, :], hprev)
                    oz = tmp.tile([128, MN, Bc], FP32, tag="oz")
                    nc.vector.tensor_scalar(oz[:], rz[:, 4:8, :], -1.0, 1.0, ALU.mult, ALU.add)
                    # h' = oz*n + u : bf16 copy feeds the next matmul sweep,
                    # f32 copy (gpsimd) is the carried state / output.
                    v = tmp.tile([128, MN, Bc], FP32, tag="v")
                    nc.vector.tensor_mul(v[:], oz[:], n[:])
                    nc.vector.tensor_add(h16[s % 2][:, :, :], v[:], u[:])
                    nc.gpsimd.tensor_add(hs_buf[:, :, s, :], v[:], u[:])

            # Prologue: x(0) -> xtA, xi(0) -> xiA, x(1) -> xtB.
            for k in range(KF):
                nc.sync.dma_start(xt_t[0][:, k, :], xt_d[k, :, 0:COLS])
            emit_xi(xt_t[0], xi_t[0])
            for k in range(KF):
                nc.sync.dma_start(xt_t[1][:, k, :], xt_d[k, :, COLS:2 * COLS])

            HINTS = (mybir.EngineType.PE, mybir.EngineType.DVE, mybir.EngineType.Activation)
            with tc.For_i(0, NPAIR, 1, hint_engines=HINTS) as i:
                # chunks c0 = 2i (buffers A=0), c1 = 2i+1 (buffers B=1)
                emit_scan(xi_t[0], hs_t[0], hs_t[1])
                emit_xi(xt_t[1], xi_t[1])          # xi(c1), fills scan-A bubbles
                for hc in range(KH):
                    nc.sync.dma_start(
                        out_d[hc, :, ds(i * (2 * COLS), COLS)], hs_t[0][:, hc, :, :]
                    )
                for k in range(KF):                 # x(c0+2) -> xtA
                    nc.sync.dma_start(
                        xt_t[0][:, k, :], xt_d[k, :, ds(i * (2 * COLS) + 2 * COLS, COLS)]
                    )
                emit_scan(xi_t[1], hs_t[1], hs_t[0])
                emit_xi(xt_t[0], xi_t[0])          # xi(c0+2), fills scan-B bubbles
                for hc in range(KH):
                    nc.sync.dma_start(
                        out_d[hc, :, ds(i * (2 * COLS) + COLS, COLS)], hs_t[1][:, hc, :, :]
                    )
                for k in range(KF):                 # x(c1+2) -> xtB
                    nc.sync.dma_start(
                        xt_t[1][:, k, :], xt_d[k, :, ds(i * (2 * COLS) + 3 * COLS, COLS)]
                    )

    nc.compile()
    return nc


_NC_CACHE = None


def _get_nc():
    global _NC_CACHE
    if _NC_CACHE is None:
        _NC_CACHE = build_nc()
    return _NC_CACHE


def _prep_core_inputs(x, W_ih, W_hh, b_ih, b_hh, layer, bg):
    xs = x[:, bg * Bc:(bg + 1) * Bc, :]                   # (T, Bc, F)
    xt = np.ascontiguousarray(np.transpose(xs, (2, 0, 1)))  # (F, T, Bc)
    xt = xt.reshape(KF, 128, T * Bc)
    xt_p = np.zeros((KF, 128, XT_COLS), np.float32)
    xt_p[:, :, :T * Bc] = xt

    wih = np.ascontiguousarray(W_ih[layer].T).reshape(KF, 128, 3 * H)
    whh = np.ascontiguousarray(W_hh[layer].T).reshape(KH, 128, 3 * H)

    bias_full = b_ih[layer].copy()
    bias_full[:2 * H] += b_hh[layer][:2 * H]
    bias = np.ascontiguousarray(bias_full.reshape(3 * H // 128, 128).T)

    bhn = b_hh[layer][2 * H:].reshape(MN, 128).T          # (128, MN)
    bhnb = np.ascontiguousarray(
        np.broadcast_to(bhn[:, :, None], (128, MN, Bc)))

    return {
        "xt": xt_p.astype(BF16),
        "wih": wih.astype(BF16),
        "whh": whh.astype(BF16),
        "iden": np.eye(128, dtype=np.float32).astype(BF16),
        "bhnb": bhnb.astype(BF16),
        "bias": bias.astype(np.float32),
    }


def run_cores(x, W_ih, W_hh, b_ih, b_hh, trace=False):
    nc = _get_nc()
    in_maps = [
        _prep_core_inputs(x, W_ih, W_hh, b_ih, b_hh, core // NBG, core % NBG)
        for core in range(NCORES)
    ]
    return run_bass_kernel_spmd(nc, in_maps, core_ids=list(range(NCORES)), trace=trace)


def assemble(results):
    out = np.zeros((T, B, H), np.float32)
    for bg in range(NBG):
        acc = None
        for layer in range(L):
            o = np.asarray(results[layer * NBG + bg]["out"], np.float32)
            hs = o.reshape(KH, 128, T, Bc).transpose(2, 3, 0, 1).reshape(T, Bc, H)
            acc = hs if acc is None else acc + hs
        out[:, bg * Bc:(bg + 1) * Bc, :] = acc / L
    return out


def kernel(x, W_ih, W_hh, b_ih, b_hh):
    x = np.asarray(x, np.float32)
    W_ih = np.asarray(W_ih, np.float32)
    W_hh = np.asarray(W_hh, np.float32)
    b_ih = np.asarray(b_ih, np.float32)
    b_hh = np.asarray(b_hh, np.float32)
    res = run_cores(x, W_ih, W_hh, b_ih, b_hh, trace=False)
    return assemble(res.results)
